# revision 22
# baseline (speedup 1.0000x reference)
"""Trainium2 Bass kernel for nn_JResCOPAttn (B=1, L=1024, D=128).

Reference computation:
    act = x @ Wl.T + bl                               # [L, E]  (E = D = 128)
    tm  = (act[:,None,:] * act[None,:,:]) @ Wlo.T + blo   # [L, L, D] (never materialized)
    tm *= (mask != 0)
    tx  = x @ Wl2.T + bl2                             # [L, D]
    y   = x + einsum('cad,ad->cd', tm, tx)
    out = LayerNorm(y) * gamma + beta

Algebraic restructuring (per output row c, channel d):
    y1[c,d] = sum_e act[c,e] * WloT[e,d] * S[c,e,d]  +  blo[d] * Z[c,d]
    S[c,e,d] = sum_a mask[c,a] * act[a,e] * tx[a,d]
    Z[c,d]   = sum_a mask[c,a] * tx[a,d]

Sharding: the e-dimension (128) is split across the 8 cores (16 e's each).
Each core computes P2[a,e,d] = act[a,e]*tx[a,d]*WloT[e,d] for its e-shard,
then S2 = maskT.T @ P2 as one large bf16 matmul (contraction over a=1024,
N=512 streams -> full PE rate; fp32 matmuls are 4x slower on TRN2).
The per-core partials y1p[c,d] = sum_{e in shard} act[c,e]*S2[c,e,d] are
summed with a ReduceScatter so core k ends up owning rows [128k, 128k+128),
where it adds the Z-term + residual and applies LayerNorm.
"""

import os
import sys

for _p in ("/opt/trn_rl_repo", "/root/.axon_site/_ro/trn_rl_repo"):
    if os.path.isdir(_p) and _p not in sys.path:
        sys.path.insert(0, _p)

import numpy as np
import ml_dtypes

import concourse.bass as bass
import concourse.tile as tile
from concourse import bacc, mybir
from concourse.bass_utils import run_bass_kernel_spmd

B, L, D = 1, 1024, 128
NCORES = 8
ESH = 16                  # e-channels per core
T = L // 128              # a-tiles = 8
CB = L // NCORES          # c-rows owned per core after ReduceScatter = 128
EPS = 1e-5
FP = mybir.dt.float32
BF = mybir.dt.bfloat16
BF_NP = ml_dtypes.bfloat16

N_DVE_J = 12              # P2-build: j < N_DVE_J on DVE, rest on gpsimd


def build_nc():
    nc = bacc.Bacc("TRN2", target_bir_lowering=False, num_devices=NCORES)

    # ---- I/O (per-core) ----
    xT    = nc.dram_tensor("xT",    [128, L], BF, kind="ExternalInput")        # x^T (d-major)
    maskT = nc.dram_tensor("maskT", [128, T, L], BF, kind="ExternalInput")     # [p,t,c] = mask[c, 128t+p]
    maskz = nc.dram_tensor("maskz", [128, T, CB], BF, kind="ExternalInput")    # own-shard columns
    WlTk  = nc.dram_tensor("WlTk",  [128, ESH], BF, kind="ExternalInput")      # Wl.T[:, e-shard]
    Wl2T  = nc.dram_tensor("Wl2T",  [128, 128], BF, kind="ExternalInput")      # Wl2.T
    WVT   = nc.dram_tensor("WVT",   [128, 128, ESH], BF, kind="ExternalInput") # WloT[e0+j, d] as [p, d, j]
    blk   = nc.dram_tensor("blk",   [128, ESH], FP, kind="ExternalInput")      # bl[e-shard] bcast
    bl2B  = nc.dram_tensor("bl2B",  [128, 128], FP, kind="ExternalInput")      # bl2 bcast
    bloB  = nc.dram_tensor("bloB",  [128, 128], FP, kind="ExternalInput")      # blo bcast
    xrow  = nc.dram_tensor("xrow",  [CB, D], FP, kind="ExternalInput")         # x rows of own c-shard
    gamB  = nc.dram_tensor("gamB",  [CB, D], FP, kind="ExternalInput")
    betB  = nc.dram_tensor("betB",  [CB, D], FP, kind="ExternalInput")
    out   = nc.dram_tensor("out",   [CB, D], FP, kind="ExternalOutput")

    Sqrt = mybir.ActivationFunctionType.Sqrt
    mult = mybir.AluOpType.mult

    with tile.TileContext(nc) as tc:
        with (
            tc.tile_pool(name="singles", bufs=1) as singles,
            tc.tile_pool(name="dram", bufs=1, space="DRAM") as dram,
            tc.tile_pool(name="gpool", bufs=2) as gpool,
            tc.tile_pool(name="h1pool", bufs=2) as h1pool,
            tc.tile_pool(name="h2pool", bufs=2) as h2pool,
            tc.tile_pool(name="h3pool", bufs=2) as h3pool,
            tc.tile_pool(name="ypool", bufs=2) as ypool,
            tc.tile_pool(name="pmain", bufs=8, space="PSUM") as pmain,
        ):
            # ---- load inputs ----
            sb_xT = singles.tile([128, L], BF)
            nc.sync.dma_start(sb_xT, xT[:, :])
            sb_WlTk = singles.tile([128, ESH], BF)
            nc.sync.dma_start(sb_WlTk, WlTk[:, :])
            sb_Wl2T = singles.tile([128, 128], BF)
            nc.sync.dma_start(sb_Wl2T, Wl2T[:, :])
            sb_WVT = singles.tile([128, 128, ESH], BF)
            nc.sync.dma_start(sb_WVT, WVT[:, :, :])
            sb_blk = singles.tile([128, ESH], FP)
            nc.sync.dma_start(sb_blk, blk[:, :])
            sb_bl2B = singles.tile([128, 128], FP)
            nc.sync.dma_start(sb_bl2B, bl2B[:, :])
            sb_maskz = singles.tile([128, T, CB], BF)
            nc.sync.dma_start(sb_maskz, maskz[:, :, :])
            sb_maskT = singles.tile([128, T, L], BF)
            for t in range(T):
                nc.sync.dma_start(sb_maskT[:, t, :], maskT[:, t, :])
            sb_bloB = singles.tile([128, 128], FP)
            nc.scalar.dma_start(sb_bloB, bloB[:, :])
            sb_xrow = singles.tile([CB, D], FP)
            nc.scalar.dma_start(sb_xrow, xrow[:, :])
            sb_gamB = singles.tile([CB, D], FP)
            nc.scalar.dma_start(sb_gamB, gamB[:, :])
            sb_betB = singles.tile([CB, D], FP)
            nc.scalar.dma_start(sb_betB, betB[:, :])

            sb_eps = singles.tile([CB, 1], FP)
            nc.vector.memset(sb_eps, EPS)

            # ---- dummy warmup collective: absorbs the cross-core rendezvous
            # and RDH stream startup while the main compute runs ----
            warm_in = dram.tile([NCORES, 16], FP)
            warm_out = dram.tile([NCORES, 16], FP)
            nc.sync.dma_start(warm_in[:, :], xrow[0:NCORES, 0:16])
            nc.gpsimd.collective_compute(
                "AllToAll",
                mybir.AluOpType.bypass,
                replica_groups=[list(range(NCORES))],
                ins=[warm_in.opt()],
                outs=[warm_out.opt()],
            )

            # ---- act_sel[a, j] (j in own e-shard) and tx[a, :] via PE ----
            act_sel = []
            tx_nat = []
            for t in range(T):
                ps = pmain.tile([128, 32, ESH], FP, tag="mm")
                xtile = sb_xT[:, t * 128:(t + 1) * 128]
                nc.tensor.matmul(ps[:, 0, 0:ESH], xtile, sb_WlTk, start=True, stop=True)
                nc.tensor.matmul(ps[:, 1:9, :], xtile, sb_Wl2T, start=True, stop=True)
                a_t = singles.tile([128, ESH], BF, name=f"act_sel{t}")
                nc.vector.tensor_add(a_t, ps[:, 0, 0:ESH], sb_blk)
                x_t = singles.tile([128, 128], BF, name=f"tx_nat{t}")
                nc.vector.tensor_add(x_t, ps[:, 1:9, :], sb_bl2B)
                act_sel.append(a_t)
                tx_nat.append(x_t)

            # ---- Z matmul for own c-shard ----
            zps = pmain.tile([128, 32, ESH], FP, tag="mm")
            for t in range(T):
                nc.tensor.matmul(
                    zps[:, 0:8, :], sb_maskz[:, t, :], tx_nat[t],
                    start=(t == 0), stop=(t == T - 1),
                )
            sb_zb = singles.tile([CB, D], FP)
            nc.vector.tensor_mul(sb_zb, zps[:, 0:8, :], sb_bloB)

            # ---- P2[t][a, d, j] = act[a,e_j] * tx[a,d]  (d-major; WloT folds
            # into the combine). Built per (q, t) chunk, q-major, so the first
            # matmul q-pass can start after one chunk and never starves.
            P2 = [singles.tile([128, 128, ESH], BF, name=f"P2_{t}") for t in range(T)]
            for q in range(4):
                for t in range(T):
                    dsl = slice(32 * q, 32 * q + 32)
                    nc.vector.tensor_mul(
                        P2[t][:, dsl, :],
                        tx_nat[t][:, dsl].unsqueeze(-1).broadcast_to((128, 32, ESH)),
                        act_sel[t][:, :].unsqueeze(1).broadcast_to((128, 32, ESH)),
                    )

            # ---- main matmuls in q-passes + per-ct combine in the last pass ----
            # S2[c, (d,j)] = sum_a mask[c,a] * P2[a, d, j]
            HF = mybir.dt.float16
            S2T = [singles.tile([128, 128, ESH], BF, name=f"S2T{ct}") for ct in range(T)]
            y1p_dram = dram.tile([L, D], HF)
            for q in range(4):
                dsl = slice(32 * q, 32 * q + 32)
                for ct in range(T):
                    ps = pmain.tile([128, 32, ESH], FP, tag="mm")
                    for t in range(T):
                        nc.tensor.matmul(
                            ps,
                            sb_maskT[:, t, ct * 128:(ct + 1) * 128],
                            P2[t][:, dsl, :],
                            start=(t == 0), stop=(t == T - 1),
                        )
                    nc.scalar.copy(S2T[ct][:, dsl, :], ps)
                    if q == 3:
                        # combine: y1p[c,d] = sum_j act[c,e_j]*WloT[e_j,d]*S2[c,d,j]
                        # alternate the two big muls between DVE and gpsimd
                        eng = nc.vector if ct % 2 == 0 else nc.gpsimd
                        gw = gpool.tile([128, 128, ESH], BF, tag="gw")
                        eng.tensor_mul(gw, S2T[ct], sb_WVT)
                        g = gpool.tile([128, 128, ESH], BF, tag="g")
                        eng.tensor_mul(
                            g, gw,
                            act_sel[ct][:, :].unsqueeze(1).broadcast_to(
                                (128, 128, ESH)
                            ),
                        )
                        h1 = h1pool.tile([128, 128, 8], BF, tag="h1")
                        nc.vector.tensor_add(h1, g[:, :, 0:8], g[:, :, 8:16])
                        h2 = h2pool.tile([128, 128, 4], BF, tag="h2")
                        nc.vector.tensor_add(h2, h1[:, :, 0:4], h1[:, :, 4:8])
                        h3 = h3pool.tile([128, 128, 2], BF, tag="h3")
                        nc.vector.tensor_add(h3, h2[:, :, 0:2], h2[:, :, 2:4])
                        y1 = ypool.tile([128, 128], HF, tag="y1")
                        nc.vector.tensor_add(y1, h3[:, :, 0], h3[:, :, 1])
                        nc.sync.dma_start(y1p_dram[ct * 128:(ct + 1) * 128, :], y1)

            # ---- AllToAll + local sum: core k gets every core's partial for
            # rows [128k, 128k+128), then adds them (faster than RDH ReduceScatter) ----
            a2a_dram = dram.tile([L, D], HF)
            nc.gpsimd.collective_compute(
                "AllToAll",
                mybir.AluOpType.bypass,
                replica_groups=[list(range(NCORES))],
                ins=[y1p_dram.opt()],
                outs=[a2a_dram.opt()],
            )
            sb_rs = singles.tile([CB, NCORES, D], HF)
            nc.sync.dma_start(
                sb_rs, a2a_dram[:, :].rearrange("(i p) d -> p i d", p=CB)
            )
            r4 = singles.tile([CB, 4, D], FP)
            nc.vector.tensor_add(r4, sb_rs[:, 0:4, :], sb_rs[:, 4:8, :])
            r2 = singles.tile([CB, 2, D], FP)
            nc.vector.tensor_add(r2, r4[:, 0:2, :], r4[:, 2:4, :])

            # ---- residual + Z + LayerNorm ----
            y_sb = singles.tile([CB, D], FP)
            nc.vector.tensor_add(y_sb, r2[:, 0, :], r2[:, 1, :])
            nc.vector.tensor_add(y_sb, y_sb, sb_xrow)
            nc.vector.tensor_add(y_sb, y_sb, sb_zb)

            stats = singles.tile([CB, nc.vector.BN_STATS_DIM], FP)
            nc.vector.bn_stats(stats, y_sb)
            mv = singles.tile([CB, 2], FP)
            nc.vector.bn_aggr(mv, stats)
            nc.vector.tensor_scalar_sub(y_sb, y_sb, mv[:, 0:1])
            sd = singles.tile([CB, 1], FP)
            nc.scalar.activation(sd, mv[:, 1:2], Sqrt, bias=sb_eps, scale=1.0)
            rstd = singles.tile([CB, 1], FP)
            nc.vector.reciprocal(rstd, sd)
            nc.vector.tensor_scalar_mul(y_sb, y_sb, rstd)
            nc.vector.tensor_mul(y_sb, y_sb, sb_gamB)
            nc.vector.tensor_add(y_sb, y_sb, sb_betB)

            nc.sync.dma_start(out[:, :], y_sb)

    return nc


_NC_CACHE = None


def _get_nc():
    global _NC_CACHE
    if _NC_CACHE is None:
        _NC_CACHE = build_nc()
        _NC_CACHE.finalize()
    return _NC_CACHE


def _prepare_in_maps(x, mask, Wl, bl, Wlo, blo, Wl2, bl2, gamma, beta):
    f32 = np.float32
    x0 = np.asarray(x, f32)[0]                       # [L, D]
    m = np.asarray(mask)[0].astype(f32)              # [L, L]  (c, a)
    xT_bf = np.ascontiguousarray(x0.T).astype(BF_NP)
    WlT = np.asarray(Wl, f32).T                      # [d, e]
    Wl2T_bf = np.ascontiguousarray(np.asarray(Wl2, f32).T).astype(BF_NP)
    WloT = np.asarray(Wlo, f32).T                    # [e, d]
    mT = m.T.reshape(T, 128, L).transpose(1, 0, 2)   # [p, t, c]
    maskT_bf = np.ascontiguousarray(mT).astype(BF_NP)
    bl_ = np.asarray(bl, f32)
    bl2B = np.ascontiguousarray(np.broadcast_to(np.asarray(bl2, f32), (128, 128)))
    bloB = np.ascontiguousarray(np.broadcast_to(np.asarray(blo, f32), (128, 128)))
    gamB = np.ascontiguousarray(np.broadcast_to(np.asarray(gamma, f32), (CB, D)))
    betB = np.ascontiguousarray(np.broadcast_to(np.asarray(beta, f32), (CB, D)))

    in_maps = []
    for k in range(NCORES):
        esl = slice(k * ESH, (k + 1) * ESH)
        blkc = slice(k * CB, (k + 1) * CB)
        mz = m[blkc, :].T.reshape(T, 128, CB).transpose(1, 0, 2)  # [p, t, c']
        in_maps.append({
            "xT": xT_bf,
            "maskT": maskT_bf,
            "maskz": np.ascontiguousarray(mz).astype(BF_NP),
            "WlTk": np.ascontiguousarray(WlT[:, esl]).astype(BF_NP),
            "Wl2T": Wl2T_bf,
            "WVT": np.ascontiguousarray(
                np.broadcast_to(WloT[esl, :].T[None, :, :], (128, 128, ESH))
            ).astype(BF_NP),
            "blk": np.ascontiguousarray(
                np.broadcast_to(bl_[esl], (128, ESH))
            ),
            "bl2B": bl2B,
            "bloB": bloB,
            "xrow": np.ascontiguousarray(x0[blkc]),
            "gamB": gamB,
            "betB": betB,
        })
    return in_maps


def kernel(x, mask, Wl, bl, Wlo, blo, Wl2, bl2, gamma, beta):
    in_maps = _prepare_in_maps(x, mask, Wl, bl, Wlo, blo, Wl2, bl2, gamma, beta)
    res = run_bass_kernel_spmd(_get_nc(), in_maps, core_ids=list(range(NCORES)))
    y = np.concatenate([res.results[k]["out"] for k in range(NCORES)], axis=0)
    return y.reshape(B, L, D).astype(np.float32)


# revision 24
# speedup vs baseline: 1.2825x; 1.2825x over previous
"""Trainium2 Bass kernel for nn_JResCOPAttn (B=1, L=1024, D=128).

Reference computation:
    act = x @ Wl.T + bl                               # [L, E]  (E = D = 128)
    tm  = (act[:,None,:] * act[None,:,:]) @ Wlo.T + blo   # [L, L, D] (never materialized)
    tm *= (mask != 0)
    tx  = x @ Wl2.T + bl2                             # [L, D]
    y   = x + einsum('cad,ad->cd', tm, tx)
    out = LayerNorm(y) * gamma + beta

Algebraic restructuring (per output row c, channel d):
    y1[c,d] = sum_e act[c,e] * WloT[e,d] * S[c,e,d]  +  blo[d] * Z[c,d]
    S[c,e,d] = sum_a mask[c,a] * act[a,e] * tx[a,d]
    Z[c,d]   = sum_a mask[c,a] * tx[a,d]

Sharding: the e-dimension (128) is split across the 8 cores (16 e's each).
Each core computes P2[a,e,d] = act[a,e]*tx[a,d]*WloT[e,d] for its e-shard,
then S2 = maskT.T @ P2 as one large bf16 matmul (contraction over a=1024,
N=512 streams -> full PE rate; fp32 matmuls are 4x slower on TRN2).
The per-core partials y1p[c,d] = sum_{e in shard} act[c,e]*S2[c,e,d] are
summed with a ReduceScatter so core k ends up owning rows [128k, 128k+128),
where it adds the Z-term + residual and applies LayerNorm.
"""

import os
import sys

for _p in ("/opt/trn_rl_repo", "/root/.axon_site/_ro/trn_rl_repo"):
    if os.path.isdir(_p) and _p not in sys.path:
        sys.path.insert(0, _p)

import numpy as np
import ml_dtypes

import concourse.bass as bass
import concourse.tile as tile
from concourse import bacc, mybir
from concourse.bass_utils import run_bass_kernel_spmd

B, L, D = 1, 1024, 128
NCORES = 8
ESH = 16                  # e-channels per core
T = L // 128              # a-tiles = 8
CB = L // NCORES          # c-rows owned per core after ReduceScatter = 128
EPS = 1e-5
FP = mybir.dt.float32
BF = mybir.dt.bfloat16
BF_NP = ml_dtypes.bfloat16

N_DVE_J = 12              # P2-build: j < N_DVE_J on DVE, rest on gpsimd


def build_nc():
    nc = bacc.Bacc("TRN2", target_bir_lowering=False, num_devices=NCORES)

    # ---- I/O (per-core) ----
    xT    = nc.dram_tensor("xT",    [128, L], BF, kind="ExternalInput")        # x^T (d-major)
    maskT = nc.dram_tensor("maskT", [128, T, L], BF, kind="ExternalInput")     # [p,t,c] = mask[c, 128t+p]
    maskz = nc.dram_tensor("maskz", [128, T, CB], BF, kind="ExternalInput")    # own-shard columns
    WlTk  = nc.dram_tensor("WlTk",  [128, ESH], BF, kind="ExternalInput")      # Wl.T[:, e-shard]
    Wl2T  = nc.dram_tensor("Wl2T",  [128, 128], BF, kind="ExternalInput")      # Wl2.T
    WVT   = nc.dram_tensor("WVT",   [128, 128, ESH], BF, kind="ExternalInput") # WloT[e0+j, d] as [p, d, j]
    blk   = nc.dram_tensor("blk",   [128, ESH], FP, kind="ExternalInput")      # bl[e-shard] bcast
    bl2B  = nc.dram_tensor("bl2B",  [128, 128], FP, kind="ExternalInput")      # bl2 bcast
    bloB  = nc.dram_tensor("bloB",  [128, 128], FP, kind="ExternalInput")      # blo bcast
    xrow  = nc.dram_tensor("xrow",  [CB, D], FP, kind="ExternalInput")         # x rows of own c-shard
    gamB  = nc.dram_tensor("gamB",  [CB, D], FP, kind="ExternalInput")
    betB  = nc.dram_tensor("betB",  [CB, D], FP, kind="ExternalInput")
    out   = nc.dram_tensor("out",   [CB, D], FP, kind="ExternalOutput")

    Sqrt = mybir.ActivationFunctionType.Sqrt
    mult = mybir.AluOpType.mult

    with tile.TileContext(nc) as tc:
        with (
            tc.tile_pool(name="singles", bufs=1) as singles,
            tc.tile_pool(name="dram", bufs=1, space="DRAM") as dram,
            tc.tile_pool(name="gpool", bufs=2) as gpool,
            tc.tile_pool(name="h1pool", bufs=2) as h1pool,
            tc.tile_pool(name="h2pool", bufs=2) as h2pool,
            tc.tile_pool(name="h3pool", bufs=2) as h3pool,
            tc.tile_pool(name="ypool", bufs=2) as ypool,
            tc.tile_pool(name="pmain", bufs=8, space="PSUM") as pmain,
        ):
            # ---- load inputs (small/critical first; big mask last) ----
            sb_xT = singles.tile([128, L], BF)
            nc.sync.dma_start(sb_xT, xT[:, :])
            sb_WlTk = singles.tile([128, ESH], BF)
            nc.sync.dma_start(sb_WlTk, WlTk[:, :])
            sb_Wl2T = singles.tile([128, 128], BF)
            nc.sync.dma_start(sb_Wl2T, Wl2T[:, :])
            sb_blk = singles.tile([128, ESH], FP)
            nc.sync.dma_start(sb_blk, blk[:, :])
            sb_bl2B = singles.tile([128, 128], FP)
            nc.sync.dma_start(sb_bl2B, bl2B[:, :])
            sb_WVT = singles.tile([128, 128, ESH], BF)
            nc.scalar.dma_start(sb_WVT, WVT[:, :, :])
            sb_maskz = singles.tile([128, T, CB], BF)
            nc.scalar.dma_start(sb_maskz, maskz[:, :, :])
            sb_maskT = singles.tile([128, T, L], BF)
            for t in range(T):
                eng = nc.sync if t % 2 == 0 else nc.scalar
                eng.dma_start(sb_maskT[:, t, :], maskT[:, t, :])
            sb_bloB = singles.tile([128, 128], FP)
            nc.scalar.dma_start(sb_bloB, bloB[:, :])
            sb_xrow = singles.tile([CB, D], FP)
            nc.scalar.dma_start(sb_xrow, xrow[:, :])
            sb_gamB = singles.tile([CB, D], FP)
            nc.scalar.dma_start(sb_gamB, gamB[:, :])
            sb_betB = singles.tile([CB, D], FP)
            nc.scalar.dma_start(sb_betB, betB[:, :])

            sb_eps = singles.tile([CB, 1], FP)
            nc.vector.memset(sb_eps, EPS)

            # ---- dummy warmup collective: absorbs the cross-core rendezvous
            # and RDH stream startup while the main compute runs ----
            warm_in = dram.tile([NCORES, 16], FP)
            warm_out = dram.tile([NCORES, 16], FP)
            nc.sync.dma_start(warm_in[:, :], xrow[0:NCORES, 0:16])
            nc.gpsimd.collective_compute(
                "AllToAll",
                mybir.AluOpType.bypass,
                replica_groups=[list(range(NCORES))],
                ins=[warm_in.opt()],
                outs=[warm_out.opt()],
            )

            # ---- act_sel[a, j] (j in own e-shard) and tx[a, :] via PE ----
            act_sel = []
            tx_nat = []
            for t in range(T):
                ps = pmain.tile([128, 32, ESH], FP, tag="mm")
                xtile = sb_xT[:, t * 128:(t + 1) * 128]
                nc.tensor.matmul(ps[:, 0, 0:ESH], xtile, sb_WlTk, start=True, stop=True)
                nc.tensor.matmul(ps[:, 1:9, :], xtile, sb_Wl2T, start=True, stop=True)
                a_t = singles.tile([128, ESH], BF, name=f"act_sel{t}")
                nc.vector.tensor_add(a_t, ps[:, 0, 0:ESH], sb_blk)
                x_t = singles.tile([128, 128], BF, name=f"tx_nat{t}")
                nc.vector.tensor_add(x_t, ps[:, 1:9, :], sb_bl2B)
                act_sel.append(a_t)
                tx_nat.append(x_t)

            # ---- Z matmul for own c-shard ----
            zps = pmain.tile([128, 32, ESH], FP, tag="mm")
            for t in range(T):
                nc.tensor.matmul(
                    zps[:, 0:8, :], sb_maskz[:, t, :], tx_nat[t],
                    start=(t == 0), stop=(t == T - 1),
                )
            sb_zb = singles.tile([CB, D], FP)
            nc.vector.tensor_mul(sb_zb, zps[:, 0:8, :], sb_bloB)

            # ---- P2[t][a, d, j] = act[a,e_j] * tx[a,d]  (d-major; WloT folds
            # into the combine). Built per (q, t) chunk, q-major, so the first
            # matmul q-pass can start after one chunk and never starves.
            P2 = [singles.tile([128, 128, ESH], BF, name=f"P2_{t}") for t in range(T)]
            for q in range(4):
                for t in range(T):
                    dsl = slice(32 * q, 32 * q + 32)
                    nc.vector.tensor_mul(
                        P2[t][:, dsl, :],
                        tx_nat[t][:, dsl].unsqueeze(-1).broadcast_to((128, 32, ESH)),
                        act_sel[t][:, :].unsqueeze(1).broadcast_to((128, 32, ESH)),
                    )

            # ---- main matmuls in q-passes; combine muls spread per (q, ct);
            # only the j-tree + DMA remain in the tail ----
            # S2[c, (d,j)] = sum_a mask[c,a] * P2[a, d, j]
            # g[ct][c, d, j] = S2[c,d,j] * WloT[e_j,d] * act[c,e_j]
            HF = mybir.dt.float16
            S2T = [singles.tile([128, 128, ESH], BF, name=f"S2T{ct}") for ct in range(T)]
            G = [singles.tile([128, 128, ESH], BF, name=f"G{ct}") for ct in range(T)]
            y1p_dram = dram.tile([L, D], HF)
            for q in range(4):
                dsl = slice(32 * q, 32 * q + 32)
                for ct in range(T):
                    ps = pmain.tile([128, 32, ESH], FP, tag="mm")
                    for t in range(T):
                        nc.tensor.matmul(
                            ps,
                            sb_maskT[:, t, ct * 128:(ct + 1) * 128],
                            P2[t][:, dsl, :],
                            start=(t == 0), stop=(t == T - 1),
                        )
                    nc.scalar.copy(S2T[ct][:, dsl, :], ps)
                    gwq = gpool.tile([128, 32, ESH], BF, tag="gwq")
                    nc.vector.tensor_mul(gwq, S2T[ct][:, dsl, :], sb_WVT[:, dsl, :])
                    nc.vector.tensor_mul(
                        G[ct][:, dsl, :], gwq,
                        act_sel[ct][:, :].unsqueeze(1).broadcast_to((128, 32, ESH)),
                    )
                    if q == 3:
                        g = G[ct]
                        h1 = h1pool.tile([128, 128, 8], BF, tag="h1")
                        nc.vector.tensor_add(h1, g[:, :, 0:8], g[:, :, 8:16])
                        h2 = h2pool.tile([128, 128, 4], BF, tag="h2")
                        nc.vector.tensor_add(h2, h1[:, :, 0:4], h1[:, :, 4:8])
                        h3 = h3pool.tile([128, 128, 2], BF, tag="h3")
                        nc.vector.tensor_add(h3, h2[:, :, 0:2], h2[:, :, 2:4])
                        y1 = ypool.tile([128, 128], HF, tag="y1")
                        nc.vector.tensor_add(y1, h3[:, :, 0], h3[:, :, 1])
                        nc.sync.dma_start(y1p_dram[ct * 128:(ct + 1) * 128, :], y1)

            # ---- AllToAll + local sum: core k gets every core's partial for
            # rows [128k, 128k+128), then adds them (faster than RDH ReduceScatter) ----
            a2a_dram = dram.tile([L, D], HF)
            nc.gpsimd.collective_compute(
                "AllToAll",
                mybir.AluOpType.bypass,
                replica_groups=[list(range(NCORES))],
                ins=[y1p_dram.opt()],
                outs=[a2a_dram.opt()],
            )
            sb_rs = singles.tile([CB, NCORES, D], HF)
            nc.sync.dma_start(
                sb_rs, a2a_dram[:, :].rearrange("(i p) d -> p i d", p=CB)
            )
            r4 = singles.tile([CB, 4, D], FP)
            nc.vector.tensor_add(r4, sb_rs[:, 0:4, :], sb_rs[:, 4:8, :])
            r2 = singles.tile([CB, 2, D], FP)
            nc.vector.tensor_add(r2, r4[:, 0:2, :], r4[:, 2:4, :])

            # ---- residual + Z + LayerNorm ----
            y_sb = singles.tile([CB, D], FP)
            nc.vector.tensor_add(y_sb, r2[:, 0, :], r2[:, 1, :])
            nc.vector.tensor_add(y_sb, y_sb, sb_xrow)
            nc.vector.tensor_add(y_sb, y_sb, sb_zb)

            stats = singles.tile([CB, nc.vector.BN_STATS_DIM], FP)
            nc.vector.bn_stats(stats, y_sb)
            mv = singles.tile([CB, 2], FP)
            nc.vector.bn_aggr(mv, stats)
            nc.vector.tensor_scalar_sub(y_sb, y_sb, mv[:, 0:1])
            sd = singles.tile([CB, 1], FP)
            nc.scalar.activation(sd, mv[:, 1:2], Sqrt, bias=sb_eps, scale=1.0)
            rstd = singles.tile([CB, 1], FP)
            nc.vector.reciprocal(rstd, sd)
            nc.vector.tensor_scalar_mul(y_sb, y_sb, rstd)
            nc.vector.tensor_mul(y_sb, y_sb, sb_gamB)
            nc.vector.tensor_add(y_sb, y_sb, sb_betB)

            nc.sync.dma_start(out[:, :], y_sb)

    return nc


_NC_CACHE = None


def _get_nc():
    global _NC_CACHE
    if _NC_CACHE is None:
        _NC_CACHE = build_nc()
        _NC_CACHE.finalize()
    return _NC_CACHE


def _prepare_in_maps(x, mask, Wl, bl, Wlo, blo, Wl2, bl2, gamma, beta):
    f32 = np.float32
    x0 = np.asarray(x, f32)[0]                       # [L, D]
    m = np.asarray(mask)[0].astype(f32)              # [L, L]  (c, a)
    xT_bf = np.ascontiguousarray(x0.T).astype(BF_NP)
    WlT = np.asarray(Wl, f32).T                      # [d, e]
    Wl2T_bf = np.ascontiguousarray(np.asarray(Wl2, f32).T).astype(BF_NP)
    WloT = np.asarray(Wlo, f32).T                    # [e, d]
    mT = m.T.reshape(T, 128, L).transpose(1, 0, 2)   # [p, t, c]
    maskT_bf = np.ascontiguousarray(mT).astype(BF_NP)
    bl_ = np.asarray(bl, f32)
    bl2B = np.ascontiguousarray(np.broadcast_to(np.asarray(bl2, f32), (128, 128)))
    bloB = np.ascontiguousarray(np.broadcast_to(np.asarray(blo, f32), (128, 128)))
    gamB = np.ascontiguousarray(np.broadcast_to(np.asarray(gamma, f32), (CB, D)))
    betB = np.ascontiguousarray(np.broadcast_to(np.asarray(beta, f32), (CB, D)))

    in_maps = []
    for k in range(NCORES):
        esl = slice(k * ESH, (k + 1) * ESH)
        blkc = slice(k * CB, (k + 1) * CB)
        mz = m[blkc, :].T.reshape(T, 128, CB).transpose(1, 0, 2)  # [p, t, c']
        in_maps.append({
            "xT": xT_bf,
            "maskT": maskT_bf,
            "maskz": np.ascontiguousarray(mz).astype(BF_NP),
            "WlTk": np.ascontiguousarray(WlT[:, esl]).astype(BF_NP),
            "Wl2T": Wl2T_bf,
            "WVT": np.ascontiguousarray(
                np.broadcast_to(WloT[esl, :].T[None, :, :], (128, 128, ESH))
            ).astype(BF_NP),
            "blk": np.ascontiguousarray(
                np.broadcast_to(bl_[esl], (128, ESH))
            ),
            "bl2B": bl2B,
            "bloB": bloB,
            "xrow": np.ascontiguousarray(x0[blkc]),
            "gamB": gamB,
            "betB": betB,
        })
    return in_maps


def kernel(x, mask, Wl, bl, Wlo, blo, Wl2, bl2, gamma, beta):
    in_maps = _prepare_in_maps(x, mask, Wl, bl, Wlo, blo, Wl2, bl2, gamma, beta)
    res = run_bass_kernel_spmd(_get_nc(), in_maps, core_ids=list(range(NCORES)))
    y = np.concatenate([res.results[k]["out"] for k in range(NCORES)], axis=0)
    return y.reshape(B, L, D).astype(np.float32)


# revision 31
# speedup vs baseline: 1.2948x; 1.0096x over previous
"""Trainium2 Bass kernel for nn_JResCOPAttn (B=1, L=1024, D=128).

Reference computation:
    act = x @ Wl.T + bl                               # [L, E]  (E = D = 128)
    tm  = (act[:,None,:] * act[None,:,:]) @ Wlo.T + blo   # [L, L, D] (never materialized)
    tm *= (mask != 0)
    tx  = x @ Wl2.T + bl2                             # [L, D]
    y   = x + einsum('cad,ad->cd', tm, tx)
    out = LayerNorm(y) * gamma + beta

Algebraic restructuring (per output row c, channel d):
    y1[c,d] = sum_e act[c,e] * WloT[e,d] * S[c,e,d]  +  blo[d] * Z[c,d]
    S[c,e,d] = sum_a mask[c,a] * act[a,e] * tx[a,d]
    Z[c,d]   = sum_a mask[c,a] * tx[a,d]

Sharding: the e-dimension (128) is split across the 8 cores (16 e's each).
Each core computes P2[a,e,d] = act[a,e]*tx[a,d]*WloT[e,d] for its e-shard,
then S2 = maskT.T @ P2 as one large bf16 matmul (contraction over a=1024,
N=512 streams -> full PE rate; fp32 matmuls are 4x slower on TRN2).
The per-core partials y1p[c,d] = sum_{e in shard} act[c,e]*S2[c,e,d] are
summed with a ReduceScatter so core k ends up owning rows [128k, 128k+128),
where it adds the Z-term + residual and applies LayerNorm.
"""

import os
import sys

for _p in ("/opt/trn_rl_repo", "/root/.axon_site/_ro/trn_rl_repo"):
    if os.path.isdir(_p) and _p not in sys.path:
        sys.path.insert(0, _p)

import numpy as np
import ml_dtypes

import concourse.bass as bass
import concourse.tile as tile
from concourse import bacc, mybir
from concourse.bass_utils import run_bass_kernel_spmd

B, L, D = 1, 1024, 128
NCORES = 8
ESH = 16                  # e-channels per core
T = L // 128              # a-tiles = 8
CB = L // NCORES          # c-rows owned per core after ReduceScatter = 128
EPS = 1e-5
FP = mybir.dt.float32
BF = mybir.dt.bfloat16
BF_NP = ml_dtypes.bfloat16

N_DVE_J = 12              # P2-build: j < N_DVE_J on DVE, rest on gpsimd


def build_nc():
    nc = bacc.Bacc("TRN2", target_bir_lowering=False, num_devices=NCORES)

    # ---- I/O (per-core) ----
    xT    = nc.dram_tensor("xT",    [128, L], BF, kind="ExternalInput")        # x^T (d-major)
    maskT = nc.dram_tensor("maskT", [128, T, L], BF, kind="ExternalInput")     # [p,t,c] = mask[c, 128t+p]
    maskz = nc.dram_tensor("maskz", [128, T, CB], BF, kind="ExternalInput")    # own-shard columns
    WlTk  = nc.dram_tensor("WlTk",  [128, ESH], BF, kind="ExternalInput")      # Wl.T[:, e-shard]
    Wl2T  = nc.dram_tensor("Wl2T",  [128, 128], BF, kind="ExternalInput")      # Wl2.T
    WVT   = nc.dram_tensor("WVT",   [128, 128, ESH], BF, kind="ExternalInput") # WloT[e0+j, d] as [p, d, j]
    blk   = nc.dram_tensor("blk",   [128, ESH], FP, kind="ExternalInput")      # bl[e-shard] bcast
    bl2B  = nc.dram_tensor("bl2B",  [128, 128], FP, kind="ExternalInput")      # bl2 bcast
    bloB  = nc.dram_tensor("bloB",  [128, 128], FP, kind="ExternalInput")      # blo bcast
    xrow  = nc.dram_tensor("xrow",  [CB, D], FP, kind="ExternalInput")         # x rows of own c-shard
    gamB  = nc.dram_tensor("gamB",  [CB, D], FP, kind="ExternalInput")
    betB  = nc.dram_tensor("betB",  [CB, D], FP, kind="ExternalInput")
    out   = nc.dram_tensor("out",   [CB, D], FP, kind="ExternalOutput")

    Sqrt = mybir.ActivationFunctionType.Sqrt
    mult = mybir.AluOpType.mult

    with tile.TileContext(nc) as tc:
        with (
            tc.tile_pool(name="singles", bufs=1) as singles,
            tc.tile_pool(name="dram", bufs=1, space="DRAM") as dram,
            tc.tile_pool(name="gpool", bufs=2) as gpool,
            tc.tile_pool(name="h1pool", bufs=2) as h1pool,
            tc.tile_pool(name="h2pool", bufs=2) as h2pool,
            tc.tile_pool(name="h3pool", bufs=2) as h3pool,
            tc.tile_pool(name="ypool", bufs=2) as ypool,
            tc.tile_pool(name="pmain", bufs=8, space="PSUM") as pmain,
        ):
            # ---- load inputs (small/critical first; big mask last) ----
            sb_xT = singles.tile([128, L], BF)
            nc.sync.dma_start(sb_xT, xT[:, :])
            sb_WlTk = singles.tile([128, ESH], BF)
            nc.sync.dma_start(sb_WlTk, WlTk[:, :])
            sb_Wl2T = singles.tile([128, 128], BF)
            nc.sync.dma_start(sb_Wl2T, Wl2T[:, :])
            sb_blk = singles.tile([128, ESH], FP)
            nc.sync.dma_start(sb_blk, blk[:, :])
            sb_bl2B = singles.tile([128, 128], FP)
            nc.sync.dma_start(sb_bl2B, bl2B[:, :])
            sb_WVT = singles.tile([128, 128, ESH], BF)
            nc.scalar.dma_start(sb_WVT, WVT[:, :, :])
            sb_maskz = singles.tile([128, T, CB], BF)
            nc.scalar.dma_start(sb_maskz, maskz[:, :, :])
            sb_maskT = singles.tile([128, T, L], BF)
            for t in range(T):
                eng = nc.sync if t % 2 == 0 else nc.scalar
                eng.dma_start(sb_maskT[:, t, :], maskT[:, t, :])
            sb_bloB = singles.tile([128, 128], FP)
            nc.scalar.dma_start(sb_bloB, bloB[:, :])
            sb_xrow = singles.tile([CB, D], FP)
            nc.scalar.dma_start(sb_xrow, xrow[:, :])
            sb_gamB = singles.tile([CB, D], FP)
            nc.scalar.dma_start(sb_gamB, gamB[:, :])
            sb_betB = singles.tile([CB, D], FP)
            nc.scalar.dma_start(sb_betB, betB[:, :])

            sb_eps = singles.tile([CB, 1], FP)
            nc.vector.memset(sb_eps, EPS)

            # ---- dummy warmup collective: absorbs the cross-core rendezvous
            # and RDH stream startup while the main compute runs ----
            warm_in = dram.tile([NCORES, 16], FP)
            warm_out = dram.tile([NCORES, 16], FP)
            nc.sync.dma_start(warm_in[:, :], xrow[0:NCORES, 0:16])
            nc.gpsimd.collective_compute(
                "AllToAll",
                mybir.AluOpType.bypass,
                replica_groups=[list(range(NCORES))],
                ins=[warm_in.opt()],
                outs=[warm_out.opt()],
            )

            # ---- act_sel[a, j] (j in own e-shard) and tx[a, :] via PE ----
            act_sel = []
            tx_nat = []
            for t in range(T):
                ps = pmain.tile([128, 32, ESH], FP, tag="mm")
                xtile = sb_xT[:, t * 128:(t + 1) * 128]
                nc.tensor.matmul(ps[:, 0, 0:ESH], xtile, sb_WlTk, start=True, stop=True)
                nc.tensor.matmul(ps[:, 1:9, :], xtile, sb_Wl2T, start=True, stop=True)
                a_t = singles.tile([128, ESH], BF, name=f"act_sel{t}")
                nc.vector.tensor_add(a_t, ps[:, 0, 0:ESH], sb_blk)
                x_t = singles.tile([128, 128], BF, name=f"tx_nat{t}")
                nc.vector.tensor_add(x_t, ps[:, 1:9, :], sb_bl2B)
                act_sel.append(a_t)
                tx_nat.append(x_t)

            # ---- Z matmul for own c-shard ----
            zps = pmain.tile([128, 32, ESH], FP, tag="mm")
            for t in range(T):
                nc.tensor.matmul(
                    zps[:, 0:8, :], sb_maskz[:, t, :], tx_nat[t],
                    start=(t == 0), stop=(t == T - 1),
                )
            sb_zb = singles.tile([CB, D], FP)
            nc.scalar.mul(sb_zb, zps[:, 0:8, :], 1.0)  # park Z in SBUF (ACT engine)

            # ---- P2[t][a, d, j] = act[a,e_j] * tx[a,d]  (d-major; WloT folds
            # into the combine). Built per (q, t) chunk, q-major, so the first
            # matmul q-pass can start after one chunk and never starves.
            P2 = [singles.tile([128, 128, ESH], BF, name=f"P2_{t}") for t in range(T)]
            for q in range(4):
                for t in range(T):
                    dsl = slice(32 * q, 32 * q + 32)
                    nc.vector.tensor_mul(
                        P2[t][:, dsl, :],
                        tx_nat[t][:, dsl].unsqueeze(-1).broadcast_to((128, 32, ESH)),
                        act_sel[t][:, :].unsqueeze(1).broadcast_to((128, 32, ESH)),
                    )

            # ---- main matmuls in q-passes; combine muls spread per (q, ct);
            # only the j-tree + DMA remain in the tail ----
            # S2[c, (d,j)] = sum_a mask[c,a] * P2[a, d, j]
            # g[ct][c, d, j] = S2[c,d,j] * WloT[e_j,d] * act[c,e_j]
            HF = mybir.dt.float16
            S2T = [singles.tile([128, 128, ESH], BF, name=f"S2T{ct}") for ct in range(T)]
            G = [singles.tile([128, 128, ESH], BF, name=f"G{ct}") for ct in range(T)]
            y1p_dram = dram.tile([L, D], HF)
            for q in range(4):
                dsl = slice(32 * q, 32 * q + 32)
                for ct in range(T):
                    ps = pmain.tile([128, 32, ESH], FP, tag="mm")
                    for i in range(T):
                        t = (ct + i) % T
                        nc.tensor.matmul(
                            ps,
                            sb_maskT[:, t, ct * 128:(ct + 1) * 128],
                            P2[t][:, dsl, :],
                            start=(i == 0), stop=(i == T - 1),
                        )
                    nc.scalar.copy(S2T[ct][:, dsl, :], ps)
                    gwq = gpool.tile([128, 32, ESH], BF, tag="gwq")
                    nc.vector.tensor_mul(gwq, S2T[ct][:, dsl, :], sb_WVT[:, dsl, :])
                    nc.vector.tensor_mul(
                        G[ct][:, dsl, :], gwq,
                        act_sel[ct][:, :].unsqueeze(1).broadcast_to((128, 32, ESH)),
                    )
                    if q == 3:
                        g = G[ct]
                        h1 = h1pool.tile([128, 128, 8], BF, tag="h1")
                        nc.vector.tensor_add(h1, g[:, :, 0:8], g[:, :, 8:16])
                        h2 = h2pool.tile([128, 128, 4], BF, tag="h2")
                        nc.vector.tensor_add(h2, h1[:, :, 0:4], h1[:, :, 4:8])
                        h3 = h3pool.tile([128, 128, 2], BF, tag="h3")
                        nc.vector.tensor_add(h3, h2[:, :, 0:2], h2[:, :, 2:4])
                        y1 = ypool.tile([128, 128], HF, tag="y1")
                        nc.vector.tensor_add(y1, h3[:, :, 0], h3[:, :, 1])
                        nc.sync.dma_start(y1p_dram[ct * 128:(ct + 1) * 128, :], y1)

            # Z*blo while the collective runs
            zw = singles.tile([CB, D], FP)
            nc.vector.tensor_mul(zw, sb_zb, sb_bloB)

            # second dummy collective keyed on the first y1p rows: keeps the
            # RDH stream warm right up to the real AllToAll
            warm2_out = dram.tile([1, D], HF)
            nc.gpsimd.collective_compute(
                "AllToAll",
                mybir.AluOpType.bypass,
                replica_groups=[list(range(NCORES))],
                ins=[y1p_dram[0:1, :].opt()],
                outs=[warm2_out.opt()],
            )

            # ---- AllToAll + local sum: core k gets every core's partial for
            # rows [128k, 128k+128), then adds them (faster than RDH ReduceScatter) ----
            a2a_dram = dram.tile([L, D], HF)
            nc.gpsimd.collective_compute(
                "AllToAll",
                mybir.AluOpType.bypass,
                replica_groups=[list(range(NCORES))],
                ins=[y1p_dram.opt()],
                outs=[a2a_dram.opt()],
            )
            sb_rs = singles.tile([CB, NCORES, D], HF)
            nc.sync.dma_start(
                sb_rs, a2a_dram[:, :].rearrange("(i p) d -> p i d", p=CB)
            )
            r4 = singles.tile([CB, 4, D], FP)
            nc.vector.tensor_add(r4, sb_rs[:, 0:4, :], sb_rs[:, 4:8, :])
            r2 = singles.tile([CB, 2, D], FP)
            nc.vector.tensor_add(r2, r4[:, 0:2, :], r4[:, 2:4, :])

            # ---- residual + Z*blo + LayerNorm ----
            y_sb = singles.tile([CB, D], FP)
            nc.vector.tensor_add(y_sb, r2[:, 0, :], r2[:, 1, :])
            nc.vector.tensor_add(y_sb, y_sb, sb_xrow)
            nc.vector.tensor_add(y_sb, y_sb, zw)

            stats = singles.tile([CB, nc.vector.BN_STATS_DIM], FP)
            nc.vector.bn_stats(stats, y_sb)
            mv = singles.tile([CB, 2], FP)
            nc.vector.bn_aggr(mv, stats)
            nc.vector.tensor_scalar_sub(y_sb, y_sb, mv[:, 0:1])
            sd = singles.tile([CB, 1], FP)
            nc.scalar.activation(sd, mv[:, 1:2], Sqrt, bias=sb_eps, scale=1.0)
            rstd = singles.tile([CB, 1], FP)
            nc.vector.reciprocal(rstd, sd)
            nc.vector.tensor_scalar_mul(y_sb, y_sb, rstd)
            nc.vector.tensor_mul(y_sb, y_sb, sb_gamB)
            nc.vector.tensor_add(y_sb, y_sb, sb_betB)

            nc.sync.dma_start(out[:, :], y_sb)

    return nc


_NC_CACHE = None


def _get_nc():
    global _NC_CACHE
    if _NC_CACHE is None:
        _NC_CACHE = build_nc()
        _NC_CACHE.finalize()
    return _NC_CACHE


def _prepare_in_maps(x, mask, Wl, bl, Wlo, blo, Wl2, bl2, gamma, beta):
    f32 = np.float32
    x0 = np.asarray(x, f32)[0]                       # [L, D]
    m = np.asarray(mask)[0].astype(f32)              # [L, L]  (c, a)
    xT_bf = np.ascontiguousarray(x0.T).astype(BF_NP)
    WlT = np.asarray(Wl, f32).T                      # [d, e]
    Wl2T_bf = np.ascontiguousarray(np.asarray(Wl2, f32).T).astype(BF_NP)
    WloT = np.asarray(Wlo, f32).T                    # [e, d]
    mT = m.T.reshape(T, 128, L).transpose(1, 0, 2)   # [p, t, c]
    maskT_bf = np.ascontiguousarray(mT).astype(BF_NP)
    bl_ = np.asarray(bl, f32)
    bl2B = np.ascontiguousarray(np.broadcast_to(np.asarray(bl2, f32), (128, 128)))
    bloB = np.ascontiguousarray(np.broadcast_to(np.asarray(blo, f32), (128, 128)))
    gamB = np.ascontiguousarray(np.broadcast_to(np.asarray(gamma, f32), (CB, D)))
    betB = np.ascontiguousarray(np.broadcast_to(np.asarray(beta, f32), (CB, D)))

    in_maps = []
    for k in range(NCORES):
        esl = slice(k * ESH, (k + 1) * ESH)
        blkc = slice(k * CB, (k + 1) * CB)
        mz = m[blkc, :].T.reshape(T, 128, CB).transpose(1, 0, 2)  # [p, t, c']
        in_maps.append({
            "xT": xT_bf,
            "maskT": maskT_bf,
            "maskz": np.ascontiguousarray(mz).astype(BF_NP),
            "WlTk": np.ascontiguousarray(WlT[:, esl]).astype(BF_NP),
            "Wl2T": Wl2T_bf,
            "WVT": np.ascontiguousarray(
                np.broadcast_to(WloT[esl, :].T[None, :, :], (128, 128, ESH))
            ).astype(BF_NP),
            "blk": np.ascontiguousarray(
                np.broadcast_to(bl_[esl], (128, ESH))
            ),
            "bl2B": bl2B,
            "bloB": bloB,
            "xrow": np.ascontiguousarray(x0[blkc]),
            "gamB": gamB,
            "betB": betB,
        })
    return in_maps


def kernel(x, mask, Wl, bl, Wlo, blo, Wl2, bl2, gamma, beta):
    in_maps = _prepare_in_maps(x, mask, Wl, bl, Wlo, blo, Wl2, bl2, gamma, beta)
    res = run_bass_kernel_spmd(_get_nc(), in_maps, core_ids=list(range(NCORES)))
    y = np.concatenate([res.results[k]["out"] for k in range(NCORES)], axis=0)
    return y.reshape(B, L, D).astype(np.float32)


# revision 32
# speedup vs baseline: 1.3533x; 1.0452x over previous
"""Trainium2 Bass kernel for nn_JResCOPAttn (B=1, L=1024, D=128).

Reference computation:
    act = x @ Wl.T + bl                               # [L, E]  (E = D = 128)
    tm  = (act[:,None,:] * act[None,:,:]) @ Wlo.T + blo   # [L, L, D] (never materialized)
    tm *= (mask != 0)
    tx  = x @ Wl2.T + bl2                             # [L, D]
    y   = x + einsum('cad,ad->cd', tm, tx)
    out = LayerNorm(y) * gamma + beta

Algebraic restructuring (per output row c, channel d):
    y1[c,d] = sum_e act[c,e] * WloT[e,d] * S[c,e,d]  +  blo[d] * Z[c,d]
    S[c,e,d] = sum_a mask[c,a] * act[a,e] * tx[a,d]
    Z[c,d]   = sum_a mask[c,a] * tx[a,d]

Sharding: the e-dimension (128) is split across the 8 cores (16 e's each).
Each core computes P2[a,e,d] = act[a,e]*tx[a,d]*WloT[e,d] for its e-shard,
then S2 = maskT.T @ P2 as one large bf16 matmul (contraction over a=1024,
N=512 streams -> full PE rate; fp32 matmuls are 4x slower on TRN2).
The per-core partials y1p[c,d] = sum_{e in shard} act[c,e]*S2[c,e,d] are
summed with a ReduceScatter so core k ends up owning rows [128k, 128k+128),
where it adds the Z-term + residual and applies LayerNorm.
"""

import os
import sys

for _p in ("/opt/trn_rl_repo", "/root/.axon_site/_ro/trn_rl_repo"):
    if os.path.isdir(_p) and _p not in sys.path:
        sys.path.insert(0, _p)

import numpy as np
import ml_dtypes

import concourse.bass as bass
import concourse.tile as tile
from concourse import bacc, mybir
from concourse.bass_utils import run_bass_kernel_spmd

B, L, D = 1, 1024, 128
NCORES = 8
ESH = 16                  # e-channels per core
T = L // 128              # a-tiles = 8
CB = L // NCORES          # c-rows owned per core after ReduceScatter = 128
EPS = 1e-5
FP = mybir.dt.float32
BF = mybir.dt.bfloat16
BF_NP = ml_dtypes.bfloat16

N_DVE_J = 12              # P2-build: j < N_DVE_J on DVE, rest on gpsimd


def build_nc():
    nc = bacc.Bacc("TRN2", target_bir_lowering=False, num_devices=NCORES)

    # ---- I/O (per-core) ----
    xT    = nc.dram_tensor("xT",    [128, L], BF, kind="ExternalInput")        # x^T (d-major)
    maskT = nc.dram_tensor("maskT", [128, T, L], BF, kind="ExternalInput")     # [p,t,c] = mask[c, 128t+p]
    maskz = nc.dram_tensor("maskz", [128, T, CB], BF, kind="ExternalInput")    # own-shard columns
    WlTk  = nc.dram_tensor("WlTk",  [128, ESH], BF, kind="ExternalInput")      # Wl.T[:, e-shard]
    Wl2T  = nc.dram_tensor("Wl2T",  [128, 128], BF, kind="ExternalInput")      # Wl2.T
    WVT   = nc.dram_tensor("WVT",   [128, 128, ESH], BF, kind="ExternalInput") # WloT[e0+j, d] as [p, d, j]
    blk   = nc.dram_tensor("blk",   [128, ESH], FP, kind="ExternalInput")      # bl[e-shard] bcast
    bl2B  = nc.dram_tensor("bl2B",  [128, 128], FP, kind="ExternalInput")      # bl2 bcast
    bloB  = nc.dram_tensor("bloB",  [128, 128], FP, kind="ExternalInput")      # blo bcast
    xrow  = nc.dram_tensor("xrow",  [CB, D], FP, kind="ExternalInput")         # x rows of own c-shard
    gamB  = nc.dram_tensor("gamB",  [CB, D], FP, kind="ExternalInput")
    betB  = nc.dram_tensor("betB",  [CB, D], FP, kind="ExternalInput")
    out   = nc.dram_tensor("out",   [CB, D], FP, kind="ExternalOutput")

    Sqrt = mybir.ActivationFunctionType.Sqrt
    mult = mybir.AluOpType.mult

    with tile.TileContext(nc) as tc:
        with (
            tc.tile_pool(name="singles", bufs=1) as singles,
            tc.tile_pool(name="dram", bufs=1, space="DRAM") as dram,
            tc.tile_pool(name="gpool", bufs=2) as gpool,
            tc.tile_pool(name="h1pool", bufs=2) as h1pool,
            tc.tile_pool(name="h2pool", bufs=2) as h2pool,
            tc.tile_pool(name="h3pool", bufs=2) as h3pool,
            tc.tile_pool(name="ypool", bufs=2) as ypool,
            tc.tile_pool(name="pmain", bufs=8, space="PSUM") as pmain,
        ):
            # ---- load inputs (small/critical first; big mask last) ----
            sb_xT = singles.tile([128, L], BF)
            nc.sync.dma_start(sb_xT, xT[:, :])
            sb_WlTk = singles.tile([128, ESH], BF)
            nc.sync.dma_start(sb_WlTk, WlTk[:, :])
            sb_Wl2T = singles.tile([128, 128], BF)
            nc.sync.dma_start(sb_Wl2T, Wl2T[:, :])
            sb_blk = singles.tile([128, ESH], FP)
            nc.sync.dma_start(sb_blk, blk[:, :])
            sb_bl2B = singles.tile([128, 128], FP)
            nc.sync.dma_start(sb_bl2B, bl2B[:, :])
            sb_WVT = singles.tile([128, 128, ESH], BF)
            nc.scalar.dma_start(sb_WVT, WVT[:, :, :])
            sb_maskz = singles.tile([128, T, CB], BF)
            nc.scalar.dma_start(sb_maskz, maskz[:, :, :])
            sb_maskT = singles.tile([128, T, L], BF)
            for t in range(T):
                eng = nc.sync if t % 2 == 0 else nc.scalar
                eng.dma_start(sb_maskT[:, t, :], maskT[:, t, :])
            sb_bloB = singles.tile([128, 128], FP)
            nc.scalar.dma_start(sb_bloB, bloB[:, :])
            sb_xrow = singles.tile([CB, D], FP)
            nc.scalar.dma_start(sb_xrow, xrow[:, :])
            sb_gamB = singles.tile([CB, D], FP)
            nc.scalar.dma_start(sb_gamB, gamB[:, :])
            sb_betB = singles.tile([CB, D], FP)
            nc.scalar.dma_start(sb_betB, betB[:, :])

            sb_eps = singles.tile([CB, 1], FP)
            nc.vector.memset(sb_eps, EPS)

            # ---- dummy warmup collective: absorbs the cross-core rendezvous
            # and RDH stream startup while the main compute runs ----
            warm_in = dram.tile([NCORES, 16], FP, tag="warm_in")
            warm_out = dram.tile([NCORES, 16], FP, tag="warm_out")
            nc.sync.dma_start(warm_in[:, :], xrow[0:NCORES, 0:16])
            nc.gpsimd.collective_compute(
                "AllToAll",
                mybir.AluOpType.bypass,
                replica_groups=[list(range(NCORES))],
                ins=[warm_in.opt()],
                outs=[warm_out.opt()],
            )

            # ---- act_sel[a, j] (j in own e-shard) and tx[a, :] via PE ----
            act_sel = []
            tx_nat = []
            for t in range(T):
                ps = pmain.tile([128, 32, ESH], FP, tag="mm")
                xtile = sb_xT[:, t * 128:(t + 1) * 128]
                nc.tensor.matmul(ps[:, 0, 0:ESH], xtile, sb_WlTk, start=True, stop=True)
                nc.tensor.matmul(ps[:, 1:9, :], xtile, sb_Wl2T, start=True, stop=True)
                a_t = singles.tile([128, ESH], BF, name=f"act_sel{t}")
                nc.vector.tensor_add(a_t, ps[:, 0, 0:ESH], sb_blk)
                x_t = singles.tile([128, 128], BF, name=f"tx_nat{t}")
                nc.vector.tensor_add(x_t, ps[:, 1:9, :], sb_bl2B)
                act_sel.append(a_t)
                tx_nat.append(x_t)

            # ---- Z matmul for own c-shard ----
            zps = pmain.tile([128, 32, ESH], FP, tag="mm")
            for t in range(T):
                nc.tensor.matmul(
                    zps[:, 0:8, :], sb_maskz[:, t, :], tx_nat[t],
                    start=(t == 0), stop=(t == T - 1),
                )
            sb_zb = singles.tile([CB, D], FP)
            nc.scalar.mul(sb_zb, zps[:, 0:8, :], 1.0)  # park Z in SBUF (ACT engine)

            # ---- P2[t][a, d, j] = act[a,e_j] * tx[a,d]  (d-major; WloT folds
            # into the combine). Built per (q, t) chunk, q-major, so the first
            # matmul q-pass can start after one chunk and never starves.
            P2 = [singles.tile([128, 128, ESH], BF, name=f"P2_{t}") for t in range(T)]
            for q in range(4):
                for t in range(T):
                    dsl = slice(32 * q, 32 * q + 32)
                    nc.vector.tensor_mul(
                        P2[t][:, dsl, :],
                        tx_nat[t][:, dsl].unsqueeze(-1).broadcast_to((128, 32, ESH)),
                        act_sel[t][:, :].unsqueeze(1).broadcast_to((128, 32, ESH)),
                    )

            # ---- main matmuls in q-passes; combine muls spread per (q, ct);
            # only the j-tree + DMA remain in the tail ----
            # S2[c, (d,j)] = sum_a mask[c,a] * P2[a, d, j]
            # g[ct][c, d, j] = S2[c,d,j] * WloT[e_j,d] * act[c,e_j]
            HF = mybir.dt.float16
            S2T = [singles.tile([128, 128, ESH], BF, name=f"S2T{ct}") for ct in range(T)]
            G = [singles.tile([128, 128, ESH], BF, name=f"G{ct}") for ct in range(T)]
            y1p_dram = dram.tile([L, D], HF, tag="y1p")
            for q in range(4):
                dsl = slice(32 * q, 32 * q + 32)
                for ct in range(T):
                    ps = pmain.tile([128, 32, ESH], FP, tag="mm")
                    for i in range(T):
                        t = (ct + i) % T
                        nc.tensor.matmul(
                            ps,
                            sb_maskT[:, t, ct * 128:(ct + 1) * 128],
                            P2[t][:, dsl, :],
                            start=(i == 0), stop=(i == T - 1),
                        )
                    nc.scalar.copy(S2T[ct][:, dsl, :], ps)
                    gwq = gpool.tile([128, 32, ESH], BF, tag="gwq")
                    nc.vector.tensor_mul(gwq, S2T[ct][:, dsl, :], sb_WVT[:, dsl, :])
                    nc.vector.tensor_mul(
                        G[ct][:, dsl, :], gwq,
                        act_sel[ct][:, :].unsqueeze(1).broadcast_to((128, 32, ESH)),
                    )
                    if q == 3:
                        g = G[ct]
                        h1 = h1pool.tile([128, 128, 8], BF, tag="h1")
                        nc.vector.tensor_add(h1, g[:, :, 0:8], g[:, :, 8:16])
                        h2 = h2pool.tile([128, 128, 4], BF, tag="h2")
                        nc.vector.tensor_add(h2, h1[:, :, 0:4], h1[:, :, 4:8])
                        h3 = h3pool.tile([128, 128, 2], BF, tag="h3")
                        nc.vector.tensor_add(h3, h2[:, :, 0:2], h2[:, :, 2:4])
                        y1 = ypool.tile([128, 128], HF, tag="y1")
                        nc.vector.tensor_add(y1, h3[:, :, 0], h3[:, :, 1])
                        nc.sync.dma_start(y1p_dram[ct * 128:(ct + 1) * 128, :], y1)

            # Z*blo while the collective runs
            zw = singles.tile([CB, D], FP)
            nc.vector.tensor_mul(zw, sb_zb, sb_bloB)

            # second dummy collective keyed on the first y1p rows: keeps the
            # RDH stream warm right up to the real AllToAll
            warm2_out = dram.tile([1, D], HF, tag="warm2_out")
            nc.gpsimd.collective_compute(
                "AllToAll",
                mybir.AluOpType.bypass,
                replica_groups=[list(range(NCORES))],
                ins=[y1p_dram[0:1, :].opt()],
                outs=[warm2_out.opt()],
            )

            # ---- AllToAll + local sum: core k gets every core's partial for
            # rows [128k, 128k+128), then adds them (faster than RDH ReduceScatter) ----
            a2a_dram = dram.tile([L, D], HF, tag="a2a")
            nc.gpsimd.collective_compute(
                "AllToAll",
                mybir.AluOpType.bypass,
                replica_groups=[list(range(NCORES))],
                ins=[y1p_dram.opt()],
                outs=[a2a_dram.opt()],
            )
            sb_rs = singles.tile([CB, NCORES, D], HF)
            nc.sync.dma_start(
                sb_rs, a2a_dram[:, :].rearrange("(i p) d -> p i d", p=CB)
            )
            r4 = singles.tile([CB, 4, D], FP)
            nc.vector.tensor_add(r4, sb_rs[:, 0:4, :], sb_rs[:, 4:8, :])
            r2 = singles.tile([CB, 2, D], FP)
            nc.vector.tensor_add(r2, r4[:, 0:2, :], r4[:, 2:4, :])

            # ---- residual + Z*blo + LayerNorm ----
            y_sb = singles.tile([CB, D], FP)
            nc.vector.tensor_add(y_sb, r2[:, 0, :], r2[:, 1, :])
            nc.vector.tensor_add(y_sb, y_sb, sb_xrow)
            nc.vector.tensor_add(y_sb, y_sb, zw)

            stats = singles.tile([CB, nc.vector.BN_STATS_DIM], FP)
            nc.vector.bn_stats(stats, y_sb)
            mv = singles.tile([CB, 2], FP)
            nc.vector.bn_aggr(mv, stats)
            nc.vector.tensor_scalar_sub(y_sb, y_sb, mv[:, 0:1])
            sd = singles.tile([CB, 1], FP)
            nc.scalar.activation(sd, mv[:, 1:2], Sqrt, bias=sb_eps, scale=1.0)
            rstd = singles.tile([CB, 1], FP)
            nc.vector.reciprocal(rstd, sd)
            nc.vector.tensor_scalar_mul(y_sb, y_sb, rstd)
            nc.vector.tensor_mul(y_sb, y_sb, sb_gamB)
            nc.vector.tensor_add(y_sb, y_sb, sb_betB)

            nc.sync.dma_start(out[:, :], y_sb)

    return nc


_NC_CACHE = None


def _get_nc():
    global _NC_CACHE
    if _NC_CACHE is None:
        _NC_CACHE = build_nc()
        _NC_CACHE.finalize()
    return _NC_CACHE


def _prepare_in_maps(x, mask, Wl, bl, Wlo, blo, Wl2, bl2, gamma, beta):
    f32 = np.float32
    x0 = np.asarray(x, f32)[0]                       # [L, D]
    m = np.asarray(mask)[0].astype(f32)              # [L, L]  (c, a)
    xT_bf = np.ascontiguousarray(x0.T).astype(BF_NP)
    WlT = np.asarray(Wl, f32).T                      # [d, e]
    Wl2T_bf = np.ascontiguousarray(np.asarray(Wl2, f32).T).astype(BF_NP)
    WloT = np.asarray(Wlo, f32).T                    # [e, d]
    mT = m.T.reshape(T, 128, L).transpose(1, 0, 2)   # [p, t, c]
    maskT_bf = np.ascontiguousarray(mT).astype(BF_NP)
    bl_ = np.asarray(bl, f32)
    bl2B = np.ascontiguousarray(np.broadcast_to(np.asarray(bl2, f32), (128, 128)))
    bloB = np.ascontiguousarray(np.broadcast_to(np.asarray(blo, f32), (128, 128)))
    gamB = np.ascontiguousarray(np.broadcast_to(np.asarray(gamma, f32), (CB, D)))
    betB = np.ascontiguousarray(np.broadcast_to(np.asarray(beta, f32), (CB, D)))

    in_maps = []
    for k in range(NCORES):
        esl = slice(k * ESH, (k + 1) * ESH)
        blkc = slice(k * CB, (k + 1) * CB)
        mz = m[blkc, :].T.reshape(T, 128, CB).transpose(1, 0, 2)  # [p, t, c']
        in_maps.append({
            "xT": xT_bf,
            "maskT": maskT_bf,
            "maskz": np.ascontiguousarray(mz).astype(BF_NP),
            "WlTk": np.ascontiguousarray(WlT[:, esl]).astype(BF_NP),
            "Wl2T": Wl2T_bf,
            "WVT": np.ascontiguousarray(
                np.broadcast_to(WloT[esl, :].T[None, :, :], (128, 128, ESH))
            ).astype(BF_NP),
            "blk": np.ascontiguousarray(
                np.broadcast_to(bl_[esl], (128, ESH))
            ),
            "bl2B": bl2B,
            "bloB": bloB,
            "xrow": np.ascontiguousarray(x0[blkc]),
            "gamB": gamB,
            "betB": betB,
        })
    return in_maps


def kernel(x, mask, Wl, bl, Wlo, blo, Wl2, bl2, gamma, beta):
    in_maps = _prepare_in_maps(x, mask, Wl, bl, Wlo, blo, Wl2, bl2, gamma, beta)
    res = run_bass_kernel_spmd(_get_nc(), in_maps, core_ids=list(range(NCORES)))
    y = np.concatenate([res.results[k]["out"] for k in range(NCORES)], axis=0)
    return y.reshape(B, L, D).astype(np.float32)
